# revision 10
# baseline (speedup 1.0000x reference)
"""Trainium2 Bass kernel for a 2-layer tanh RNN (B=64, T=512, H=512, V=25, OUT=1000).

Strategy: data-parallel over batch (8 per core, 8 cores). Each core runs:
  1. embed: one-hot matmul xw0^T = table^T @ OH  (biases b_ih0+b_hh0 folded into table)
  2. layer-0 scan in transposed-h layout: hT[n,B] = tanh(sum_k Whh0T[k,n].T @ hT[k,B] + xw0T)
  3. xw1 chunk matmuls (interleaved as PE filler): xw1T = Wih1T.T @ h1T + (b_ih1+b_hh1)
  4. layer-1 scan (same shape), 5. projection out = h2_last @ Wout.T + b_out.

All matmuls bf16 (fp32 is 4 cycles/column on the PE; bf16 is 1 and gets FWL on
weight loads). PSUM accumulation is fp32. Verified numerically: ~3e-3 max rel err.
"""
import os
import sys

for _p in ("/opt/trn_rl_repo", "/root/.axon_site/_ro/trn_rl_repo"):
    if os.path.isdir(_p):
        if _p not in sys.path:
            sys.path.insert(0, _p)
        break

import numpy as np
import ml_dtypes

import concourse.bass as bass
import concourse.mybir as mybir
from concourse.tile import TileContext
from concourse.bass_utils import run_bass_kernel_spmd

BF = ml_dtypes.bfloat16
N_CORES = 8
B, H, V, OUT = 64, 512, 25, 1000
BL = B // N_CORES            # batch per core
KC = H // 128                # 4 k/n chunks
CHUNK_T = 64                 # timesteps per xw1 filler chunk


def _split_sync_waits(nc, limit=1):
    """This walrus build rejects instructions with >1 sync-wait command.
    Hoist excess on_wait entries onto preceding NoOps on the same engine."""
    cnt = 0
    for fn in nc.m.functions:
        for blk in fn.blocks:
            out = []
            for inst in blk.instructions:
                si = inst.sync_info
                if si is not None:
                    waits = list(si.on_wait)
                    if len(waits) > limit:
                        extra, keep = waits[:-limit], waits[-limit:]
                        for j in range(0, len(extra), limit):
                            cnt += 1
                            nop = mybir.InstNoOp(name=f"waitnop-{cnt}", ins=[], outs=[])
                            nop.engine = inst.engine
                            nop.sync_info = mybir.SyncInfo(
                                on_wait=extra[j:j + limit], on_update=[])
                            out.append(nop)
                        inst.sync_info = mybir.SyncInfo(
                            on_wait=keep, on_update=list(si.on_update))
                out.append(inst)
            blk.instructions = out
    return cnt


def _emit_scan(nc, tc, T, wt, xw_st, h_st, ppool, fill_fn):
    """One RNN layer scan. wt: 4 weight tiles [128,512] (k-part, n-cols).
    xw_st/h_st: pairs of store tiles [128, T*16]; step t occupies cols
    [16t,16t+16), chunk k in pair-half (k&1)*8. fill_fn(cc) emits filler work
    after the last step that completes h columns for chunk cc."""
    TANH = mybir.ActivationFunctionType.Tanh

    def sl(st, t, k):           # [128, 8] slice of h/xw store for chunk k
        return st[k >> 1][:, t * 16 + (k & 1) * 8: t * 16 + (k & 1) * 8 + 8]

    def pair_sl(st, t, half):   # [128, 16] step slice for pair half (0=n01,1=n23)
        return st[half][:, t * 16: t * 16 + 16]

    # t = 0: h0 = tanh(xw_0)
    for half in range(2):
        nc.scalar.activation(pair_sl(h_st, 0, half), pair_sl(xw_st, 0, half), TANH)
    for t in range(1, T):
        for half in range(2):
            ps = ppool.tile([128, 16], mybir.dt.float32, name="scanps",
                            tag="scanps%d" % half)
            first = True
            for k in range(KC):
                for n in (2 * half, 2 * half + 1):
                    # start=True clears has_written for the WHOLE bank, so only
                    # the first matmul of the step may set it; later regions'
                    # first writes overwrite via cleared has_written bits.
                    nc.tensor.matmul(
                        ps[:, (n & 1) * 8:(n & 1) * 8 + 8],
                        wt[k][:, n * 128:(n + 1) * 128],
                        sl(h_st, t - 1, k),
                        start=first, stop=(k == KC - 1),
                        skip_group_check=True)
                    first = False
            nc.vector.tensor_add(out=ps[:], in0=ps[:], in1=pair_sl(xw_st, t, half))
            nc.scalar.activation(pair_sl(h_st, t, half), ps[:], TANH)
        if fill_fn is not None and (t + 1) % CHUNK_T == 0:
            fill_fn((t + 1) // CHUNK_T - 1)


def _build(T, reps=1):
    nc = bass.Bass()
    d = mybir.dt
    oh_d = nc.declare_dram_parameter("oh", [V, BL * T], d.bfloat16, isOutput=False)
    tab_d = nc.declare_dram_parameter("tab", [V, H], d.bfloat16, isOutput=False)
    whh0_d = nc.declare_dram_parameter("whh0", [H, H], d.bfloat16, isOutput=False)
    wih1_d = nc.declare_dram_parameter("wih1", [H, H], d.bfloat16, isOutput=False)
    whh1_d = nc.declare_dram_parameter("whh1", [H, H], d.bfloat16, isOutput=False)
    bias1_d = nc.declare_dram_parameter("bias1", [H, 1], d.float32, isOutput=False)
    wout_d = nc.declare_dram_parameter("wout", [H, OUT], d.bfloat16, isOutput=False)
    bout_d = nc.declare_dram_parameter("bout", [BL, OUT], d.float32, isOutput=False)
    y_d = nc.declare_dram_parameter("y", [BL, OUT], d.float32, isOutput=True)

    n_cc = (BL * T) // 512      # 512-column chunks over (t, b) columns

    with TileContext(nc) as tc:
        with (
            tc.tile_pool(name="cst", bufs=1) as cst,
            tc.tile_pool(name="store", bufs=1) as store,
            tc.tile_pool(name="work", bufs=2) as work,
            tc.tile_pool(name="pbig", bufs=2, space="PSUM") as pbig,
            tc.tile_pool(name="pscan", bufs=2, space="PSUM") as pscan,
        ):
            # --- load constants ---
            oh = cst.tile([V, BL * T], d.bfloat16)
            nc.sync.dma_start(out=oh[:], in_=oh_d[:, :])
            tab = cst.tile([V, H], d.bfloat16)
            nc.sync.dma_start(out=tab[:], in_=tab_d[:, :])
            w0, w1i, w1h, wo = [], [], [], []
            for k in range(KC):
                r = slice(k * 128, (k + 1) * 128)
                for lst, src, cols, wn in ((w0, whh0_d, H, "w0"),
                                           (w1i, wih1_d, H, "w1i"),
                                           (w1h, whh1_d, H, "w1h"),
                                           (wo, wout_d, OUT, "wo")):
                    tl = cst.tile([128, cols], d.bfloat16,
                                  name=f"{wn}_{k}", tag=f"{wn}_{k}")
                    nc.sync.dma_start(out=tl[:], in_=src[r, :])
                    lst.append(tl)
            b1 = cst.tile([128, KC], d.float32)
            for k in range(KC):
                nc.sync.dma_start(out=b1[:, k:k + 1],
                                  in_=bias1_d[k * 128:(k + 1) * 128, :])
            bo = cst.tile([BL, OUT], d.float32)
            nc.sync.dma_start(out=bo[:], in_=bout_d[:, :])

            from contextlib import ExitStack as _ES
            _loop_ctx = _ES()
            if reps > 1:
                _loop_ctx.enter_context(tc.For_i(0, reps, 1))

            # --- stores: pairs of [128, T*16] (bf16) ---
            def st_pair(nm):
                return [store.tile([128, T * 16], d.bfloat16, name=f"{nm}{i}",
                                   tag=f"{nm}{i}") for i in range(2)]
            xw0_st, h1_st, xw1_st, h2_st = (st_pair(n) for n in
                                            ("xw0", "h1", "xw1", "h2"))

            def chunk_view(st_pair_, n, cc):
                # [128, 64, 8] view: timesteps of 512-col chunk cc, n-chunk n
                tpc = 512 // BL  # timesteps per chunk
                return (st_pair_[n >> 1][:]
                        .rearrange("p (t x) -> p t x", x=16)
                        [:, cc * tpc:(cc + 1) * tpc, (n & 1) * 8:(n & 1) * 8 + 8])

            # --- embed: xw0T = tab.T @ OH ---
            for cc in range(n_cc):
                for n in range(KC):
                    ps = pbig.tile([128, 512], d.float32, name="pbig", tag="pbig")
                    nc.tensor.matmul(ps[:], tab[:, n * 128:(n + 1) * 128],
                                     oh[:, cc * 512:(cc + 1) * 512],
                                     start=True, stop=True)
                    eng = nc.vector if (n % 2 == 0) else nc.scalar
                    if eng is nc.vector:
                        nc.vector.tensor_copy(
                            out=chunk_view(xw0_st, n, cc),
                            in_=ps[:].rearrange("p (t x) -> p t x", x=8))
                    else:
                        nc.scalar.activation(
                            chunk_view(xw0_st, n, cc),
                            ps[:].rearrange("p (t x) -> p t x", x=8),
                            mybir.ActivationFunctionType.Copy)

            # --- xw1 filler for chunk cc (ran inside L0 scan) ---
            def xw1_fill(cc):
                for n in range(KC):
                    ps = pbig.tile([128, 512], d.float32, name="pbig", tag="pbig")
                    for k in range(KC):
                        nc.tensor.matmul(
                            ps[:], w1i[k][:, n * 128:(n + 1) * 128],
                            chunk_view(h1_st, k, cc),
                            start=(k == 0), stop=(k == KC - 1))
                    nc.vector.tensor_scalar_add(
                        out=chunk_view(xw1_st, n, cc),
                        in0=ps[:].rearrange("p (t x) -> p t x", x=8),
                        scalar1=b1[:, n:n + 1])

            # --- scans ---
            _emit_scan(nc, tc, T, w0, xw0_st, h1_st, pscan, xw1_fill)
            _emit_scan(nc, tc, T, w1h, xw1_st, h2_st, pscan, None)

            # --- projection: y = h2[T-1] @ Wout.T + b_out ---
            outs = []
            for c0 in range(0, OUT, 512):
                w = min(512, OUT - c0)
                ps = pbig.tile([128, 512], d.float32, name="pbig", tag="pbig")
                for k in range(KC):
                    lhsT = h2_st[k >> 1][:, (T - 1) * 16 + (k & 1) * 8:
                                         (T - 1) * 16 + (k & 1) * 8 + 8]
                    nc.tensor.matmul(ps[:BL, :w], lhsT, wo[k][:, c0:c0 + w],
                                     start=(k == 0), stop=(k == KC - 1))
                outs.append((c0, w, ps))
            y_sb = work.tile([BL, OUT], d.float32)
            for c0, w, ps in outs:
                nc.vector.tensor_add(out=y_sb[:, c0:c0 + w], in0=ps[:BL, :w],
                                     in1=bo[:, c0:c0 + w])
            nc.sync.dma_start(out=y_d[:, :], in_=y_sb[:])
            _loop_ctx.close()

    _split_sync_waits(nc)
    return nc


_NC_CACHE = {}


def _get_nc(T):
    if T not in _NC_CACHE:
        _NC_CACHE[T] = _build(T)
    return _NC_CACHE[T]


def _build_null(T):
    """Same I/O signature, no compute — for overhead calibration in test.py."""
    nc = bass.Bass()
    d = mybir.dt
    nc.declare_dram_parameter("oh", [V, BL * T], d.bfloat16, isOutput=False)
    nc.declare_dram_parameter("tab", [V, H], d.bfloat16, isOutput=False)
    nc.declare_dram_parameter("whh0", [H, H], d.bfloat16, isOutput=False)
    nc.declare_dram_parameter("wih1", [H, H], d.bfloat16, isOutput=False)
    nc.declare_dram_parameter("whh1", [H, H], d.bfloat16, isOutput=False)
    nc.declare_dram_parameter("bias1", [H, 1], d.float32, isOutput=False)
    nc.declare_dram_parameter("wout", [H, OUT], d.bfloat16, isOutput=False)
    bout_d = nc.declare_dram_parameter("bout", [BL, OUT], d.float32, isOutput=False)
    y_d = nc.declare_dram_parameter("y", [BL, OUT], d.float32, isOutput=True)
    with TileContext(nc) as tc:
        with tc.tile_pool(name="work", bufs=1) as work:
            t = work.tile([BL, OUT], mybir.dt.float32)
            nc.sync.dma_start(out=t[:], in_=bout_d[:, :])
            nc.sync.dma_start(out=y_d[:, :], in_=t[:])
    _split_sync_waits(nc)
    return nc


def _prep_in_maps(x, inp):
    x = np.asarray(x)
    T = x.shape[1]
    tab = (np.asarray(inp["W_ih0"]).T + np.asarray(inp["b_ih0"])
           + np.asarray(inp["b_hh0"])).astype(BF)
    whh0 = np.ascontiguousarray(np.asarray(inp["W_hh0"]).T).astype(BF)
    wih1 = np.ascontiguousarray(np.asarray(inp["W_ih1"]).T).astype(BF)
    whh1 = np.ascontiguousarray(np.asarray(inp["W_hh1"]).T).astype(BF)
    bias1 = (np.asarray(inp["b_ih1"]) + np.asarray(inp["b_hh1"])
             ).astype(np.float32)[:, None]
    wout = np.ascontiguousarray(np.asarray(inp["W_out"]).T).astype(BF)
    bout = np.broadcast_to(np.asarray(inp["b_out"], np.float32), (BL, OUT)).copy()

    in_maps = []
    for c in range(N_CORES):
        xs = x[c * BL:(c + 1) * BL]              # [BL, T]
        # one-hot [V, BL*T], column order (t, b)
        cols = np.ascontiguousarray(xs.T).reshape(-1).astype(np.int64)  # t-major
        oh = np.zeros((V, BL * T), dtype=BF)
        oh[cols, np.arange(BL * T)] = BF(1.0)
        in_maps.append({
            "oh": oh, "tab": tab, "whh0": whh0, "wih1": wih1, "whh1": whh1,
            "bias1": bias1, "wout": wout, "bout": bout,
        })
    return in_maps


def kernel(x, W_ih0, W_hh0, b_ih0, b_hh0, W_ih1, W_hh1, b_ih1, b_hh1, W_out, b_out):
    x = np.asarray(x)
    T = x.shape[1]
    nc = _get_nc(T)
    in_maps = _prep_in_maps(x, dict(
        W_ih0=W_ih0, b_ih0=b_ih0, b_hh0=b_hh0, W_hh0=W_hh0, W_ih1=W_ih1,
        b_ih1=b_ih1, b_hh1=b_hh1, W_hh1=W_hh1, W_out=W_out, b_out=b_out))
    res = run_bass_kernel_spmd(nc, in_maps, core_ids=list(range(N_CORES)))
    out = np.concatenate([res.results[c]["y"] for c in range(N_CORES)], axis=0)
    return out.astype(np.float32)


# revision 26
# speedup vs baseline: 1.4522x; 1.4522x over previous
"""Trainium2 Bass kernel for a 2-layer tanh RNN (B=64, T=512, H=512, V=25, OUT=1000).

Strategy: data-parallel over batch (8 per core, 8 cores). Each core runs:
  1. embed: one-hot matmul xw0^T = table^T @ OH  (biases b_ih0+b_hh0 folded into table)
  2. layer-0 scan in transposed-h layout: hT[n,B] = tanh(sum_k Whh0T[k,n].T @ hT[k,B] + xw0T)
  3. xw1 chunk matmuls (interleaved as PE filler): xw1T = Wih1T.T @ h1T + (b_ih1+b_hh1)
  4. layer-1 scan (same shape), 5. projection out = h2_last @ Wout.T + b_out.

All matmuls bf16 (fp32 is 4 cycles/column on the PE; bf16 is 1 and gets FWL on
weight loads). PSUM accumulation is fp32. Verified numerically: ~3e-3 max rel err.
"""
import os
import sys

for _p in ("/opt/trn_rl_repo", "/root/.axon_site/_ro/trn_rl_repo"):
    if os.path.isdir(_p):
        if _p not in sys.path:
            sys.path.insert(0, _p)
        break

import numpy as np
import ml_dtypes

import concourse.bass as bass
import concourse.mybir as mybir
from concourse.tile import TileContext
from concourse.bass_utils import run_bass_kernel_spmd

BF = ml_dtypes.bfloat16
N_CORES = 8
B, H, V, OUT = 64, 512, 25, 1000
BL = B // N_CORES            # batch per core
KC = H // 128                # 4 k/n chunks
CHUNK_T = 64                 # timesteps per xw1 filler chunk


def _split_sync_waits(nc, limit=1):
    """This walrus build rejects instructions with >1 sync-wait command.
    Hoist excess on_wait entries onto preceding NoOps on the same engine."""
    cnt = 0
    for fn in nc.m.functions:
        for blk in fn.blocks:
            out = []
            for inst in blk.instructions:
                si = inst.sync_info
                if si is not None:
                    waits = list(si.on_wait)
                    if len(waits) > limit:
                        extra, keep = waits[:-limit], waits[-limit:]
                        for j in range(0, len(extra), limit):
                            cnt += 1
                            nop = mybir.InstNoOp(name=f"waitnop-{cnt}", ins=[], outs=[])
                            nop.engine = inst.engine
                            nop.sync_info = mybir.SyncInfo(
                                on_wait=extra[j:j + limit], on_update=[])
                            out.append(nop)
                        inst.sync_info = mybir.SyncInfo(
                            on_wait=keep, on_update=list(si.on_update))
                out.append(inst)
            blk.instructions = out
    return cnt


SCAN_MODE = os.environ.get("RNN_SCAN_MODE", "full")  # full | mm_only | tail_only
SW = 32  # step width in store columns: chunk k at offset 8k


def _make_step_emitter(nc, ppool, wt, xw_st, h_st, tag):
    """Returns emit(t) for one RNN layer scan chain. wt: 4 weight tiles
    [128,512] (k-part, n-cols). xw_st/h_st: store tiles [128, T*SW]; step t
    occupies cols [SW*t, SW*t+SW), chunk k at sub-offset 8k.

    Step structure: DVE prefills xw into the psum bank (off the critical
    chain), 16 matmuls accumulate on top via has_written semantics (all
    start=False once the bank has been seeded), one ACT tanh evacuates to the
    h store. Critical chain per step is MM->ACT->MM only; interleaving two
    chains hides each ACT under the other chain's matmul block."""
    TANH = mybir.ActivationFunctionType.Tanh
    PBUFS = 2

    def sl(st, t, k):           # [128, 8] slice of h/xw store for chunk k
        return st[:, t * SW + 8 * k: t * SW + 8 * k + 8]

    def step_sl(st, t):         # [128, SW] step slice
        return st[:, t * SW: t * SW + SW]

    def emit(t):
        if t == 0:  # h0 = tanh(xw_0)
            nc.scalar.activation(step_sl(h_st, 0), step_sl(xw_st, 0), TANH)
            return
        boot = t <= PBUFS  # first use of each psum slot: seed has_written
        ps = ppool.tile([128, SW], mybir.dt.float32, name="scanps", tag=tag)
        if not boot and SCAN_MODE != "mm_only":
            nc.vector.tensor_copy(out=ps[:], in_=step_sl(xw_st, t))
        if SCAN_MODE != "tail_only":
            first = boot
            for k in range(KC):
                for n in range(KC):
                    # start=True clears has_written for the WHOLE bank, so only
                    # the first matmul of a fresh bank may set it; afterwards
                    # all matmuls accumulate onto the DVE-prefilled xw values.
                    nc.tensor.matmul(
                        ps[:, 8 * n:8 * n + 8],
                        wt[k][:, n * 128:(n + 1) * 128],
                        sl(h_st, t - 1, k),
                        start=first, stop=(k == KC - 1),
                        skip_group_check=True)
                    first = False
        if SCAN_MODE != "mm_only":
            if boot:
                nc.vector.tensor_add(out=ps[:], in0=ps[:], in1=step_sl(xw_st, t))
            nc.scalar.activation(step_sl(h_st, t), ps[:], TANH)

    return emit


def _build(T, reps=1):
    nc = bass.Bass()
    d = mybir.dt
    oh_d = nc.declare_dram_parameter("oh", [V, BL * T], d.bfloat16, isOutput=False)
    tab_d = nc.declare_dram_parameter("tab", [V, H], d.bfloat16, isOutput=False)
    whh0_d = nc.declare_dram_parameter("whh0", [H, H], d.bfloat16, isOutput=False)
    wih1_d = nc.declare_dram_parameter("wih1", [H, H], d.bfloat16, isOutput=False)
    whh1_d = nc.declare_dram_parameter("whh1", [H, H], d.bfloat16, isOutput=False)
    bias1_d = nc.declare_dram_parameter("bias1", [H, 1], d.float32, isOutput=False)
    wout_d = nc.declare_dram_parameter("wout", [H, OUT], d.bfloat16, isOutput=False)
    bout_d = nc.declare_dram_parameter("bout", [BL, OUT], d.float32, isOutput=False)
    y_d = nc.declare_dram_parameter("y", [BL, OUT], d.float32, isOutput=True)

    n_cc = (BL * T) // 512      # 512-column chunks over (t, b) columns

    with TileContext(nc) as tc:
        with (
            tc.tile_pool(name="cst", bufs=1) as cst,
            tc.tile_pool(name="store", bufs=1) as store,
            tc.tile_pool(name="work", bufs=2) as work,
            tc.tile_pool(name="pbig", bufs=2, space="PSUM") as pbig,
            tc.tile_pool(name="pscan", bufs=2, space="PSUM") as pscan,
        ):
            # --- load constants ---
            oh = cst.tile([V, BL * T], d.bfloat16)
            nc.sync.dma_start(out=oh[:], in_=oh_d[:, :])
            tab = cst.tile([V, H], d.bfloat16)
            nc.sync.dma_start(out=tab[:], in_=tab_d[:, :])
            w0, w1i, w1h, wo = [], [], [], []
            for k in range(KC):
                r = slice(k * 128, (k + 1) * 128)
                for lst, src, cols, wn in ((w0, whh0_d, H, "w0"),
                                           (w1i, wih1_d, H, "w1i"),
                                           (w1h, whh1_d, H, "w1h"),
                                           (wo, wout_d, OUT, "wo")):
                    tl = cst.tile([128, cols], d.bfloat16,
                                  name=f"{wn}_{k}", tag=f"{wn}_{k}")
                    nc.sync.dma_start(out=tl[:], in_=src[r, :])
                    lst.append(tl)
            b1 = cst.tile([128, KC], d.float32)
            for k in range(KC):
                nc.sync.dma_start(out=b1[:, k:k + 1],
                                  in_=bias1_d[k * 128:(k + 1) * 128, :])
            bo = cst.tile([BL, OUT], d.float32)
            nc.sync.dma_start(out=bo[:], in_=bout_d[:, :])

            from contextlib import ExitStack as _ES
            _loop_ctx = _ES()
            if reps > 1:
                _loop_ctx.enter_context(tc.For_i(0, reps, 1))

            # --- stores: single tiles [128, T*SW] (bf16) ---
            def st_tile(nm):
                return store.tile([128, T * SW], d.bfloat16, name=nm, tag=nm)
            xw0_st, h1_st, xw1_st, h2_st = (st_tile(n) for n in
                                            ("xw0", "h1", "xw1", "h2"))

            def chunk_view(st, n, cc, tpc=512 // BL):
                # [128, tpc, 8] view: timesteps of chunk cc, n-chunk n
                return (st[:]
                        .rearrange("p (t x) -> p t x", x=SW)
                        [:, cc * tpc:(cc + 1) * tpc, n * 8:n * 8 + 8])

            # --- embed chunk: xw0T = tab.T @ OH (512 cols of (t,b)) ---
            def embed_fill(cc):
                for n in range(KC):
                    ps = pbig.tile([128, 512], d.float32, name="pbig", tag="pbig")
                    nc.tensor.matmul(ps[:], tab[:, n * 128:(n + 1) * 128],
                                     oh[:, cc * 512:(cc + 1) * 512],
                                     start=True, stop=True)
                    nc.vector.tensor_copy(
                        out=chunk_view(xw0_st, n, cc),
                        in_=ps[:].rearrange("p (t x) -> p t x", x=8))

            # --- xw1 filler for a CHUNK_T-timestep chunk cc (inside L0 scan) ---
            def xw1_fill(cc):
                w = CHUNK_T * BL
                for n in range(KC):
                    ps = pbig.tile([128, 512], d.float32, name="pbig", tag="pbig")
                    for k in range(KC):
                        nc.tensor.matmul(
                            ps[:, :w], w1i[k][:, n * 128:(n + 1) * 128],
                            chunk_view(h1_st, k, cc, CHUNK_T),
                            start=(k == 0), stop=(k == KC - 1))
                    nc.vector.tensor_scalar_add(
                        out=chunk_view(xw1_st, n, cc, CHUNK_T),
                        in0=ps[:, :w].rearrange("p (t x) -> p t x", x=8),
                        scalar1=b1[:, n:n + 1])

            # --- interleaved dual-chain scans: L0 runs one chunk ahead of L1;
            # each chain's ACT hides under the other chain's matmuls ---
            em0 = _make_step_emitter(nc, pscan, w0, xw0_st, h1_st, "ps0")
            em1 = _make_step_emitter(nc, pscan, w1h, xw1_st, h2_st, "ps1")
            embed_cols = 512 // BL              # timesteps per embed chunk
            embed_fill(0)
            for s in range(CHUNK_T):            # prologue: L0 chunk 0 solo,
                em0(s)                          # embed chunks 1.. interleaved
                if (s + 1) % 8 == 0:
                    cc_e = (s + 1) // 8
                    if cc_e < n_cc:
                        embed_fill(cc_e)
            for cc_e in range(max(1, CHUNK_T // 8), n_cc):
                embed_fill(cc_e)
            xw1_fill(0)
            for i in range(T - CHUNK_T):        # steady: L0 step 64+i || L1 step i
                em0(CHUNK_T + i)
                em1(i)
                if (CHUNK_T + i + 1) % CHUNK_T == 0:
                    cc = (CHUNK_T + i + 1) // CHUNK_T - 1  # L0 chunk just done
                    if cc < T // CHUNK_T:
                        xw1_fill(cc)
            for i in range(T - CHUNK_T, T):     # epilogue: L1 tail solo
                em1(i)

            # --- projection: y = h2[T-1] @ Wout.T + b_out ---
            outs = []
            for c0 in range(0, OUT, 512):
                w = min(512, OUT - c0)
                ps = pbig.tile([128, 512], d.float32, name="pbig", tag="pbig")
                for k in range(KC):
                    lhsT = h2_st[:, (T - 1) * SW + 8 * k:(T - 1) * SW + 8 * k + 8]
                    nc.tensor.matmul(ps[:BL, :w], lhsT, wo[k][:, c0:c0 + w],
                                     start=(k == 0), stop=(k == KC - 1))
                outs.append((c0, w, ps))
            y_sb = work.tile([BL, OUT], d.float32)
            for c0, w, ps in outs:
                nc.vector.tensor_add(out=y_sb[:, c0:c0 + w], in0=ps[:BL, :w],
                                     in1=bo[:, c0:c0 + w])
            nc.sync.dma_start(out=y_d[:, :], in_=y_sb[:])
            _loop_ctx.close()

    _split_sync_waits(nc)
    return nc


_NC_CACHE = {}


def _get_nc(T):
    if T not in _NC_CACHE:
        _NC_CACHE[T] = _build(T)
    return _NC_CACHE[T]


def _build_null(T):
    """Same I/O signature, no compute — for overhead calibration in test.py."""
    nc = bass.Bass()
    d = mybir.dt
    nc.declare_dram_parameter("oh", [V, BL * T], d.bfloat16, isOutput=False)
    nc.declare_dram_parameter("tab", [V, H], d.bfloat16, isOutput=False)
    nc.declare_dram_parameter("whh0", [H, H], d.bfloat16, isOutput=False)
    nc.declare_dram_parameter("wih1", [H, H], d.bfloat16, isOutput=False)
    nc.declare_dram_parameter("whh1", [H, H], d.bfloat16, isOutput=False)
    nc.declare_dram_parameter("bias1", [H, 1], d.float32, isOutput=False)
    nc.declare_dram_parameter("wout", [H, OUT], d.bfloat16, isOutput=False)
    bout_d = nc.declare_dram_parameter("bout", [BL, OUT], d.float32, isOutput=False)
    y_d = nc.declare_dram_parameter("y", [BL, OUT], d.float32, isOutput=True)
    with TileContext(nc) as tc:
        with tc.tile_pool(name="work", bufs=1) as work:
            t = work.tile([BL, OUT], mybir.dt.float32)
            nc.sync.dma_start(out=t[:], in_=bout_d[:, :])
            nc.sync.dma_start(out=y_d[:, :], in_=t[:])
    _split_sync_waits(nc)
    return nc


def _prep_in_maps(x, inp):
    x = np.asarray(x)
    T = x.shape[1]
    tab = (np.asarray(inp["W_ih0"]).T + np.asarray(inp["b_ih0"])
           + np.asarray(inp["b_hh0"])).astype(BF)
    whh0 = np.ascontiguousarray(np.asarray(inp["W_hh0"]).T).astype(BF)
    wih1 = np.ascontiguousarray(np.asarray(inp["W_ih1"]).T).astype(BF)
    whh1 = np.ascontiguousarray(np.asarray(inp["W_hh1"]).T).astype(BF)
    bias1 = (np.asarray(inp["b_ih1"]) + np.asarray(inp["b_hh1"])
             ).astype(np.float32)[:, None]
    wout = np.ascontiguousarray(np.asarray(inp["W_out"]).T).astype(BF)
    bout = np.broadcast_to(np.asarray(inp["b_out"], np.float32), (BL, OUT)).copy()

    in_maps = []
    for c in range(N_CORES):
        xs = x[c * BL:(c + 1) * BL]              # [BL, T]
        # one-hot [V, BL*T], column order (t, b)
        cols = np.ascontiguousarray(xs.T).reshape(-1).astype(np.int64)  # t-major
        oh = np.zeros((V, BL * T), dtype=BF)
        oh[cols, np.arange(BL * T)] = BF(1.0)
        in_maps.append({
            "oh": oh, "tab": tab, "whh0": whh0, "wih1": wih1, "whh1": whh1,
            "bias1": bias1, "wout": wout, "bout": bout,
        })
    return in_maps


def kernel(x, W_ih0, W_hh0, b_ih0, b_hh0, W_ih1, W_hh1, b_ih1, b_hh1, W_out, b_out):
    x = np.asarray(x)
    T = x.shape[1]
    nc = _get_nc(T)
    in_maps = _prep_in_maps(x, dict(
        W_ih0=W_ih0, b_ih0=b_ih0, b_hh0=b_hh0, W_hh0=W_hh0, W_ih1=W_ih1,
        b_ih1=b_ih1, b_hh1=b_hh1, W_hh1=W_hh1, W_out=W_out, b_out=b_out))
    res = run_bass_kernel_spmd(nc, in_maps, core_ids=list(range(N_CORES)))
    out = np.concatenate([res.results[c]["y"] for c in range(N_CORES)], axis=0)
    return out.astype(np.float32)


# revision 33
# speedup vs baseline: 1.7747x; 1.2221x over previous
"""Trainium2 Bass kernel for a 2-layer tanh RNN (B=64, T=512, H=512, V=25, OUT=1000).

Strategy: data-parallel over batch (8 per core, 8 cores). Each core runs:
  1. embed: one-hot matmul xw0^T = table^T @ OH  (biases b_ih0+b_hh0 folded into table)
  2. layer-0 scan in transposed-h layout: hT[n,B] = tanh(sum_k Whh0T[k,n].T @ hT[k,B] + xw0T)
  3. xw1 chunk matmuls (interleaved as PE filler): xw1T = Wih1T.T @ h1T + (b_ih1+b_hh1)
  4. layer-1 scan (same shape), 5. projection out = h2_last @ Wout.T + b_out.

All matmuls bf16 (fp32 is 4 cycles/column on the PE; bf16 is 1 and gets FWL on
weight loads). PSUM accumulation is fp32. Verified numerically: ~3e-3 max rel err.
"""
import os
import sys

for _p in ("/opt/trn_rl_repo", "/root/.axon_site/_ro/trn_rl_repo"):
    if os.path.isdir(_p):
        if _p not in sys.path:
            sys.path.insert(0, _p)
        break

import numpy as np
import ml_dtypes

import concourse.bass as bass
import concourse.mybir as mybir
from concourse.tile import TileContext
from concourse.bass_utils import run_bass_kernel_spmd

BF = ml_dtypes.bfloat16
N_CORES = 8
B, H, V, OUT = 64, 512, 25, 1000
BL = B // N_CORES            # batch per core
KC = H // 128                # 4 k/n chunks
CHUNK_T = 64                 # timesteps per xw1 filler chunk


def _split_sync_waits(nc, limit=1):
    """This walrus build rejects instructions with >1 sync-wait command.
    Hoist excess on_wait entries onto preceding NoOps on the same engine."""
    cnt = 0
    for fn in nc.m.functions:
        for blk in fn.blocks:
            out = []
            for inst in blk.instructions:
                si = inst.sync_info
                if si is not None:
                    waits = list(si.on_wait)
                    if len(waits) > limit:
                        extra, keep = waits[:-limit], waits[-limit:]
                        for j in range(0, len(extra), limit):
                            cnt += 1
                            nop = mybir.InstNoOp(name=f"waitnop-{cnt}", ins=[], outs=[])
                            nop.engine = inst.engine
                            nop.sync_info = mybir.SyncInfo(
                                on_wait=extra[j:j + limit], on_update=[])
                            out.append(nop)
                        inst.sync_info = mybir.SyncInfo(
                            on_wait=keep, on_update=list(si.on_update))
                out.append(inst)
            blk.instructions = out
    return cnt


SCAN_MODE = os.environ.get("RNN_SCAN_MODE", "full")  # full | mm_only | tail_only
SW = 32  # step width in store columns: chunk k at offset 8k


# odd deg-9 tanh approx on [-2.2, 2.2] (max abs err ~1.1e-3):
# tanh(x) ~ C4*x*((((t+PA)*t+PB)*t+PC)*t+PD), t=x^2
# (layer-1 pre-activations satisfy |z| <= ~1.7; clamp guards outliers)
PCLAMP, C4 = 2.2, 0.000856407769166798
PA, PB = -15.14051368946318, 95.07694378151602
PC, PD = -351.55283507942045, 1160.6054841689622


def _make_step_emitter(nc, ppool, wt, xw_st, h_st, tag, wpool=None, last_t=None):
    """Returns emit(t) for one RNN layer scan chain. wt: 4 weight tiles
    [128,512] (k-part, n-cols). xw_st/h_st: store tiles [128, T*SW]; step t
    occupies cols [SW*t, SW*t+SW), chunk k at sub-offset 8k.

    Step structure: DVE prefills xw into the psum bank (off the critical
    chain), 16 matmuls accumulate on top via has_written semantics (all
    start=False once the bank has been seeded), one ACT tanh evacuates to the
    h store. Critical chain per step is MM->ACT->MM only; interleaving two
    chains hides each ACT under the other chain's matmul block."""
    TANH = mybir.ActivationFunctionType.Tanh
    PBUFS = 2

    def sl(st, t, k):           # [128, 8] slice of h/xw store for chunk k
        return st[:, t * SW + 8 * k: t * SW + 8 * k + 8]

    def step_sl(st, t):         # [128, SW] step slice
        return st[:, t * SW: t * SW + SW]

    def emit(t):
        if t == 0:  # h0 = tanh(xw_0)
            nc.scalar.activation(step_sl(h_st, 0), step_sl(xw_st, 0), TANH)
            return
        boot = t <= PBUFS  # first use of each psum slot: seed has_written
        ps = ppool.tile([128, SW], mybir.dt.float32, name="scanps", tag=tag)
        if not boot and SCAN_MODE != "mm_only":
            nc.vector.tensor_copy(out=ps[:], in_=step_sl(xw_st, t))
        if SCAN_MODE != "tail_only":
            first = boot
            for k in range(KC):
                for n in range(KC):
                    # start=True clears has_written for the WHOLE bank, so only
                    # the first matmul of a fresh bank may set it; afterwards
                    # all matmuls accumulate onto the DVE-prefilled xw values.
                    nc.tensor.matmul(
                        ps[:, 8 * n:8 * n + 8],
                        wt[k][:, n * 128:(n + 1) * 128],
                        sl(h_st, t - 1, k),
                        start=first, stop=(k == KC - 1),
                        skip_group_check=True)
                    first = False
        if SCAN_MODE != "mm_only":
            if boot:
                nc.vector.tensor_add(out=ps[:], in0=ps[:], in1=step_sl(xw_st, t))
            if SCAN_MODE == "dve_tail":      # timing probe: tail off ACT engine
                nc.vector.tensor_copy(out=step_sl(h_st, t), in_=ps[:])
            elif wpool is not None and not boot and t != last_t:
                # DVE polynomial tanh: keeps this chain's tail off the ACT
                # engine so the two chains' tails run on different engines.
                mu, ad = mybir.AluOpType.mult, mybir.AluOpType.add
                xc = wpool.tile([128, SW], mybir.dt.float32, name="xc", tag=tag + "xc")
                nc.vector.tensor_scalar(out=xc[:], in0=ps[:], scalar1=PCLAMP,
                                        scalar2=-PCLAMP,
                                        op0=mybir.AluOpType.min,
                                        op1=mybir.AluOpType.max)
                tt = wpool.tile([128, SW], mybir.dt.float32, name="tt", tag=tag + "tt")
                nc.vector.tensor_tensor(out=tt[:], in0=xc[:], in1=xc[:], op=mu)
                u1 = wpool.tile([128, SW], mybir.dt.float32, name="u1", tag=tag + "u1")
                nc.vector.scalar_tensor_tensor(out=u1[:], in0=tt[:], scalar=PA,
                                               in1=tt[:], op0=ad, op1=mu)
                nc.vector.scalar_tensor_tensor(out=u1[:], in0=u1[:], scalar=PB,
                                               in1=tt[:], op0=ad, op1=mu)
                nc.vector.scalar_tensor_tensor(out=u1[:], in0=u1[:], scalar=PC,
                                               in1=tt[:], op0=ad, op1=mu)
                nc.vector.scalar_tensor_tensor(out=u1[:], in0=u1[:], scalar=PD,
                                               in1=xc[:], op0=ad, op1=mu)
                nc.vector.tensor_scalar_mul(out=step_sl(h_st, t), in0=u1[:],
                                            scalar1=C4)
            else:
                nc.scalar.activation(step_sl(h_st, t), ps[:], TANH)

    return emit


def _build(T, reps=1):
    nc = bass.Bass()
    d = mybir.dt
    oh_d = nc.declare_dram_parameter("oh", [V, BL * T], d.bfloat16, isOutput=False)
    tab_d = nc.declare_dram_parameter("tab", [V, H], d.bfloat16, isOutput=False)
    whh0_d = nc.declare_dram_parameter("whh0", [H, H], d.bfloat16, isOutput=False)
    wih1_d = nc.declare_dram_parameter("wih1", [H, H], d.bfloat16, isOutput=False)
    whh1_d = nc.declare_dram_parameter("whh1", [H, H], d.bfloat16, isOutput=False)
    bias1_d = nc.declare_dram_parameter("bias1", [H, 1], d.float32, isOutput=False)
    wout_d = nc.declare_dram_parameter("wout", [H, OUT], d.bfloat16, isOutput=False)
    bout_d = nc.declare_dram_parameter("bout", [BL, OUT], d.float32, isOutput=False)
    y_d = nc.declare_dram_parameter("y", [BL, OUT], d.float32, isOutput=True)

    n_cc = (BL * T) // 512      # 512-column chunks over (t, b) columns

    with TileContext(nc) as tc:
        with (
            tc.tile_pool(name="cst", bufs=1) as cst,
            tc.tile_pool(name="store", bufs=1) as store,
            tc.tile_pool(name="work", bufs=2) as work,
            tc.tile_pool(name="pbig", bufs=2, space="PSUM") as pbig,
            tc.tile_pool(name="pscan", bufs=2, space="PSUM") as pscan,
        ):
            # --- load constants ---
            oh = cst.tile([V, BL * T], d.bfloat16)
            nc.sync.dma_start(out=oh[:], in_=oh_d[:, :])
            tab = cst.tile([V, H], d.bfloat16)
            nc.sync.dma_start(out=tab[:], in_=tab_d[:, :])
            w0, w1i, w1h, wo = [], [], [], []
            for k in range(KC):
                r = slice(k * 128, (k + 1) * 128)
                for lst, src, cols, wn in ((w0, whh0_d, H, "w0"),
                                           (w1i, wih1_d, H, "w1i"),
                                           (w1h, whh1_d, H, "w1h"),
                                           (wo, wout_d, OUT, "wo")):
                    tl = cst.tile([128, cols], d.bfloat16,
                                  name=f"{wn}_{k}", tag=f"{wn}_{k}")
                    nc.sync.dma_start(out=tl[:], in_=src[r, :])
                    lst.append(tl)
            b1 = cst.tile([128, KC], d.float32)
            for k in range(KC):
                nc.sync.dma_start(out=b1[:, k:k + 1],
                                  in_=bias1_d[k * 128:(k + 1) * 128, :])
            bo = cst.tile([BL, OUT], d.float32)
            nc.sync.dma_start(out=bo[:], in_=bout_d[:, :])

            from contextlib import ExitStack as _ES
            _loop_ctx = _ES()
            if reps > 1:
                _loop_ctx.enter_context(tc.For_i(0, reps, 1))

            # --- stores: single tiles [128, T*SW] (bf16) ---
            def st_tile(nm):
                return store.tile([128, T * SW], d.bfloat16, name=nm, tag=nm)
            xw0_st, h1_st, xw1_st, h2_st = (st_tile(n) for n in
                                            ("xw0", "h1", "xw1", "h2"))

            def chunk_view(st, n, cc, tpc=512 // BL):
                # [128, tpc, 8] view: timesteps of chunk cc, n-chunk n
                return (st[:]
                        .rearrange("p (t x) -> p t x", x=SW)
                        [:, cc * tpc:(cc + 1) * tpc, n * 8:n * 8 + 8])

            # --- embed chunk: xw0T = tab.T @ OH (512 cols of (t,b)) ---
            def embed_fill(cc):
                for n in range(KC):
                    ps = pbig.tile([128, 512], d.float32, name="pbig", tag="pbig")
                    nc.tensor.matmul(ps[:], tab[:, n * 128:(n + 1) * 128],
                                     oh[:, cc * 512:(cc + 1) * 512],
                                     start=True, stop=True)
                    nc.vector.tensor_copy(
                        out=chunk_view(xw0_st, n, cc),
                        in_=ps[:].rearrange("p (t x) -> p t x", x=8))

            # --- xw1 filler for a CHUNK_T-timestep chunk cc (inside L0 scan) ---
            def xw1_fill(cc):
                w = CHUNK_T * BL
                for n in range(KC):
                    ps = pbig.tile([128, 512], d.float32, name="pbig", tag="pbig")
                    for k in range(KC):
                        nc.tensor.matmul(
                            ps[:, :w], w1i[k][:, n * 128:(n + 1) * 128],
                            chunk_view(h1_st, k, cc, CHUNK_T),
                            start=(k == 0), stop=(k == KC - 1))
                    nc.vector.tensor_scalar_add(
                        out=chunk_view(xw1_st, n, cc, CHUNK_T),
                        in0=ps[:, :w].rearrange("p (t x) -> p t x", x=8),
                        scalar1=b1[:, n:n + 1])

            # --- interleaved dual-chain scans: L0 runs one chunk ahead of L1;
            # each chain's ACT hides under the other chain's matmuls ---
            em0 = _make_step_emitter(nc, pscan, w0, xw0_st, h1_st, "ps0")
            em1 = _make_step_emitter(nc, pscan, w1h, xw1_st, h2_st, "ps1")
            embed_cols = 512 // BL              # timesteps per embed chunk
            embed_fill(0)
            for s in range(CHUNK_T):            # prologue: L0 chunk 0 solo,
                em0(s)                          # embed chunks 1.. interleaved
                if (s + 1) % 8 == 0:
                    cc_e = (s + 1) // 8
                    if cc_e < n_cc:
                        embed_fill(cc_e)
            for cc_e in range(max(1, CHUNK_T // 8), n_cc):
                embed_fill(cc_e)
            xw1_fill(0)
            for i in range(T - CHUNK_T):        # steady: L0 step 64+i || L1 step i
                em0(CHUNK_T + i)
                em1(i)
                if (CHUNK_T + i + 1) % CHUNK_T == 0:
                    cc = (CHUNK_T + i + 1) // CHUNK_T - 1  # L0 chunk just done
                    if cc < T // CHUNK_T:
                        xw1_fill(cc)
            for i in range(T - CHUNK_T, T):     # epilogue: L1 tail solo
                em1(i)

            # --- projection: y = h2[T-1] @ Wout.T + b_out ---
            outs = []
            for c0 in range(0, OUT, 512):
                w = min(512, OUT - c0)
                ps = pbig.tile([128, 512], d.float32, name="pbig", tag="pbig")
                for k in range(KC):
                    lhsT = h2_st[:, (T - 1) * SW + 8 * k:(T - 1) * SW + 8 * k + 8]
                    nc.tensor.matmul(ps[:BL, :w], lhsT, wo[k][:, c0:c0 + w],
                                     start=(k == 0), stop=(k == KC - 1))
                outs.append((c0, w, ps))
            y_sb = work.tile([BL, OUT], d.float32)
            for c0, w, ps in outs:
                nc.vector.tensor_add(out=y_sb[:, c0:c0 + w], in0=ps[:BL, :w],
                                     in1=bo[:, c0:c0 + w])
            nc.sync.dma_start(out=y_d[:, :], in_=y_sb[:])
            _loop_ctx.close()

    _split_sync_waits(nc)
    return nc


_NC_CACHE = {}


def _get_nc(T):
    if T not in _NC_CACHE:
        _NC_CACHE[T] = _build(T)
    return _NC_CACHE[T]


def _build_null(T):
    """Same I/O signature, no compute — for overhead calibration in test.py."""
    nc = bass.Bass()
    d = mybir.dt
    nc.declare_dram_parameter("oh", [V, BL * T], d.bfloat16, isOutput=False)
    nc.declare_dram_parameter("tab", [V, H], d.bfloat16, isOutput=False)
    nc.declare_dram_parameter("whh0", [H, H], d.bfloat16, isOutput=False)
    nc.declare_dram_parameter("wih1", [H, H], d.bfloat16, isOutput=False)
    nc.declare_dram_parameter("whh1", [H, H], d.bfloat16, isOutput=False)
    nc.declare_dram_parameter("bias1", [H, 1], d.float32, isOutput=False)
    nc.declare_dram_parameter("wout", [H, OUT], d.bfloat16, isOutput=False)
    bout_d = nc.declare_dram_parameter("bout", [BL, OUT], d.float32, isOutput=False)
    y_d = nc.declare_dram_parameter("y", [BL, OUT], d.float32, isOutput=True)
    with TileContext(nc) as tc:
        with tc.tile_pool(name="work", bufs=1) as work:
            t = work.tile([BL, OUT], mybir.dt.float32)
            nc.sync.dma_start(out=t[:], in_=bout_d[:, :])
            nc.sync.dma_start(out=y_d[:, :], in_=t[:])
    _split_sync_waits(nc)
    return nc


def _prep_in_maps(x, inp):
    x = np.asarray(x)
    T = x.shape[1]
    tab = (np.asarray(inp["W_ih0"]).T + np.asarray(inp["b_ih0"])
           + np.asarray(inp["b_hh0"])).astype(BF)
    whh0 = np.ascontiguousarray(np.asarray(inp["W_hh0"]).T).astype(BF)
    wih1 = np.ascontiguousarray(np.asarray(inp["W_ih1"]).T).astype(BF)
    whh1 = np.ascontiguousarray(np.asarray(inp["W_hh1"]).T).astype(BF)
    bias1 = (np.asarray(inp["b_ih1"]) + np.asarray(inp["b_hh1"])
             ).astype(np.float32)[:, None]
    wout = np.ascontiguousarray(np.asarray(inp["W_out"]).T).astype(BF)
    bout = np.broadcast_to(np.asarray(inp["b_out"], np.float32), (BL, OUT)).copy()

    in_maps = []
    for c in range(N_CORES):
        xs = x[c * BL:(c + 1) * BL]              # [BL, T]
        # one-hot [V, BL*T], column order (t, b)
        cols = np.ascontiguousarray(xs.T).reshape(-1).astype(np.int64)  # t-major
        oh = np.zeros((V, BL * T), dtype=BF)
        oh[cols, np.arange(BL * T)] = BF(1.0)
        in_maps.append({
            "oh": oh, "tab": tab, "whh0": whh0, "wih1": wih1, "whh1": whh1,
            "bias1": bias1, "wout": wout, "bout": bout,
        })
    return in_maps


def kernel(x, W_ih0, W_hh0, b_ih0, b_hh0, W_ih1, W_hh1, b_ih1, b_hh1, W_out, b_out):
    x = np.asarray(x)
    T = x.shape[1]
    nc = _get_nc(T)
    in_maps = _prep_in_maps(x, dict(
        W_ih0=W_ih0, b_ih0=b_ih0, b_hh0=b_hh0, W_hh0=W_hh0, W_ih1=W_ih1,
        b_ih1=b_ih1, b_hh1=b_hh1, W_hh1=W_hh1, W_out=W_out, b_out=b_out))
    res = run_bass_kernel_spmd(nc, in_maps, core_ids=list(range(N_CORES)))
    out = np.concatenate([res.results[c]["y"] for c in range(N_CORES)], axis=0)
    return out.astype(np.float32)
